# revision 1
# baseline (speedup 1.0000x reference)
"""Hyperbolic GNN classifier on 8 Trainium2 NeuronCores (Bass/Tile).

Node-sharded: each core owns 8192 of 65536 nodes. Per layer: local
transform, AllGather of a 257-col table (256 feats + ||row||^2), then a
16-step Mobius left-fold aggregation using indirect-DMA row gathers
(one 128-row gather per tile per step; GpSimd does only these).

Fold state per row is (alpha, nA2): acc = alpha*A with nA2 = ||A||^2.
Per fold step, full-width work is two DVE passes (diff = A - y, AXPY
A += beta*y) plus one ACT square-accum of diff; the dot <A,y> is
recovered as (nA2 + y2 - ||A-y||^2)/2 with y2 gathered as table col
256. All per-row coefficients run as [128, G]-batched small ops.
"""

import os

import numpy as np

import concourse.bass as bass
import concourse.bacc as bacc
import concourse.mybir as mybir
import concourse.tile as tile
from concourse.bass_utils import run_bass_kernel_spmd
from concourse.masks import make_identity

f32 = mybir.dt.float32
i32 = mybir.dt.int32
OP = mybir.AluOpType
AF = mybir.ActivationFunctionType

NCORES = 8
N = 65536
KDEG = 16
DIN = 512
DH = 256
DOUT = 64
B = 64
NSH = N // NCORES          # 8192 nodes per core
P = 128
TILES = NSH // P           # 64 node tiles per core
ST = 264                   # table row stride (floats); col 256 = ||row||^2
G = int(os.environ.get("KG", "32"))  # tiles per aggregation group
NG = TILES // G
TG = 4                     # tiles per matmul/chain sub-batch (PSUM limited)
EPS = 1e-7
MAXN = 1.0 - 1e-5
NORM = float(KDEG) ** -0.5  # 0.25

TRACE = False
LAST_RESULT = None
STAGE = int(os.environ.get("KSTAGE", "5"))
KDEBUG = bool(int(os.environ.get("KDEBUG", "0")))
LAST_EXEC_NS = None
_NC_CACHE = {}


def _atanh_over(nc, pool, yn, g, p=P):
    """Returns s = arctanh(yn)/yn as a [p, g] tile. yn clipped already."""
    V = nc.vector
    S = nc.scalar
    a = pool.tile([p, g], f32, tag=f"ch_a{p}")
    V.tensor_scalar(out=a[:], in0=yn[:], scalar1=1.0, scalar2=None,
                    op0=OP.add)
    bm = pool.tile([p, g], f32, tag=f"ch_b{p}")
    V.tensor_scalar(out=bm[:], in0=yn[:], scalar1=-1.0, scalar2=1.0,
                    op0=OP.mult, op1=OP.add)
    rb = pool.tile([p, g], f32, tag=f"ch_c{p}")
    V.reciprocal(rb[:], bm[:])
    v = pool.tile([p, g], f32, tag=f"ch_a2{p}")
    V.tensor_tensor(out=v[:], in0=a[:], in1=rb[:], op=OP.mult)
    w = pool.tile([p, g], f32, tag=f"ch_b2{p}")
    S.activation(w[:], v[:], AF.Ln)
    q = pool.tile([p, g], f32, tag=f"ch_c2{p}")
    V.reciprocal(q[:], yn[:])
    s = pool.tile([p, g], f32, tag=f"ch_d{p}")
    V.tensor_tensor(out=s[:], in0=w[:], in1=q[:], op=OP.mult)
    sh = pool.tile([p, g], f32, tag=f"ch_e{p}")
    V.tensor_scalar(out=sh[:], in0=s[:], scalar1=0.5, scalar2=None,
                    op0=OP.mult)
    return sh


def _fold_chain(nc, pool, d2, alpha, nA2, y2ap, y2_is_col, g):
    """Mobius-fold coefficients on a [128, g] batch.

    d2 = ||A - y||^2 per row; state (alpha, nA2); y2ap = ||y||^2 as a
    [128, g] AP or a per-partition [128, 1] column (y2_is_col).
    Returns (beta, alpha_new, nA2_new).
    """
    V = nc.vector
    S = nc.scalar

    def _with_y2(out, in0, op):
        if y2_is_col:
            V.tensor_scalar(out=out[:], in0=in0[:], scalar1=y2ap,
                            scalar2=None, op0=op)
        else:
            V.tensor_tensor(out=out[:], in0=in0[:], in1=y2ap, op=op)

    # S2 = nA2 + y2 - d2  (= 2 * <A, y>)
    e1 = pool.tile([P, g], f32, tag="fc_e1")
    V.tensor_tensor(out=e1[:], in0=nA2[:], in1=d2[:], op=OP.subtract)
    S2 = pool.tile([P, g], f32, tag="fc_S2")
    _with_y2(S2, e1, OP.add)
    # xy = alpha * <A,y> = 0.5 * alpha * S2
    ah = pool.tile([P, g], f32, tag="fc_ah")
    V.tensor_scalar(out=ah[:], in0=alpha[:], scalar1=0.5, scalar2=None,
                    op0=OP.mult)
    xy = pool.tile([P, g], f32, tag="fc_xy")
    V.tensor_tensor(out=xy[:], in0=S2[:], in1=ah[:], op=OP.mult)
    # a2 = ||acc||^2 = alpha^2 * nA2
    aa = pool.tile([P, g], f32, tag="fc_aa")
    V.tensor_tensor(out=aa[:], in0=alpha[:], in1=alpha[:], op=OP.mult)
    a2 = pool.tile([P, g], f32, tag="fc_a2")
    V.tensor_tensor(out=a2[:], in0=aa[:], in1=nA2[:], op=OP.mult)
    u = pool.tile([P, g], f32, tag="fc_u")
    S.activation(u[:], xy[:], AF.Copy, bias=1.0, scale=2.0)  # 2xy+1
    ca = pool.tile([P, g], f32, tag="fc_ca")
    _with_y2(ca, u, OP.add)
    t0 = pool.tile([P, g], f32, tag="fc_t0")
    _with_y2(t0, a2, OP.mult)
    den = pool.tile([P, g], f32, tag="fc_den")
    V.tensor_tensor(out=den[:], in0=u[:], in1=t0[:], op=OP.add)
    denc = pool.tile([P, g], f32, tag="fc_denc")
    V.tensor_scalar(out=denc[:], in0=den[:], scalar1=EPS, scalar2=None,
                    op0=OP.max)
    d = pool.tile([P, g], f32, tag="fc_d")
    V.reciprocal(d[:], denc[:])
    s1 = pool.tile([P, g], f32, tag="fc_s1")
    V.tensor_tensor(out=s1[:], in0=ca[:], in1=d[:], op=OP.mult)
    cy = pool.tile([P, g], f32, tag="fc_cy")
    S.activation(cy[:], a2[:], AF.Copy, bias=1.0, scale=-1.0)  # 1-a2
    s2 = pool.tile([P, g], f32, tag="fc_s2")
    V.tensor_tensor(out=s2[:], in0=cy[:], in1=d[:], op=OP.mult)
    alpha_n = pool.tile([P, g], f32, tag="fc_al")
    V.tensor_tensor(out=alpha_n[:], in0=s1[:], in1=alpha[:], op=OP.mult)
    winv = pool.tile([P, g], f32, tag="fc_wi")
    V.reciprocal(winv[:], alpha_n[:])
    beta = pool.tile([P, g], f32, tag="fc_be")
    V.tensor_tensor(out=beta[:], in0=s2[:], in1=winv[:], op=OP.mult)
    # nA2' = nA2 + beta*S2 + beta^2*y2
    m1 = pool.tile([P, g], f32, tag="fc_m1")
    V.tensor_tensor(out=m1[:], in0=beta[:], in1=S2[:], op=OP.mult)
    m2 = pool.tile([P, g], f32, tag="fc_m2")
    V.tensor_tensor(out=m2[:], in0=nA2[:], in1=m1[:], op=OP.add)
    m3 = pool.tile([P, g], f32, tag="fc_m3")
    V.tensor_tensor(out=m3[:], in0=beta[:], in1=beta[:], op=OP.mult)
    m4 = pool.tile([P, g], f32, tag="fc_m4")
    _with_y2(m4, m3, OP.mult)
    nA2_n = pool.tile([P, g], f32, tag="fc_n2n")
    V.tensor_tensor(out=nA2_n[:], in0=m2[:], in1=m4[:], op=OP.add)
    return beta, alpha_n, nA2_n


def _fold_step(nc, pools, acc, yin_of, y2ap, y2_is_col, alpha, nA2, g,
               width=DH):
    """One fold step for a group: diff + ACT sq-accum dots, chain, AXPYs."""
    V = nc.vector
    S = nc.scalar
    sb, ch = pools["sb"], pools["ch"]
    d2 = ch.tile([P, g], f32, tag="d2")
    for t in range(g):
        diff = sb.tile([P, width], f32, tag="diff")
        V.tensor_tensor(out=diff[:], in0=acc[:, t, 0:width], in1=yin_of(t),
                        op=OP.subtract)
        sqs = sb.tile([P, width], f32, tag="sqs")
        S.activation(sqs[:], diff[:], AF.Square,
                     accum_out=d2[:, t : t + 1])
    beta, alpha_n, nA2_n = _fold_chain(nc, ch, d2, alpha, nA2, y2ap,
                                       y2_is_col, g)
    for t in range(g):
        V.scalar_tensor_tensor(
            out=acc[:, t, 0:width], in0=yin_of(t),
            scalar=beta[:, t : t + 1], in1=acc[:, t, 0:width],
            op0=OP.mult, op1=OP.add)
    return alpha_n, nA2_n


def _aggregate(nc, pools, idx_sb, table, bias_row, bn2col, group_cb,
               do_bias=True):
    """16-step Mobius fold over gathered rows (+ optional bias fold).

    group_cb(gi, acc, alpha, nA2): acc [P, G, 257], alpha/nA2 [P, G].
    """
    V = nc.vector
    GP = nc.gpsimd
    ch, yb, ac = pools["ch"], pools["yb"], pools["ac"]

    for gi in range(NG):
        acc = ac.tile([P, G, 257], f32, tag="acc")
        for t in range(G):
            GP.indirect_dma_start(
                out=acc[:, t, :], out_offset=None, in_=table[:],
                in_offset=bass.IndirectOffsetOnAxis(
                    ap=idx_sb[:, 0, gi * G + t : gi * G + t + 1], axis=0))
        alpha = ch.tile([P, G], f32, tag="st_alpha")
        nc.vector.memset(alpha[:], 1.0)
        nA2 = ch.tile([P, G], f32, tag="st_nA2")
        V.tensor_copy(out=nA2[:], in_=acc[:, :, 256])

        for k in range(1, KDEG):
            ybuf = yb.tile([P, G, 257], f32, tag="ybuf")
            for t in range(G):
                GP.indirect_dma_start(
                    out=ybuf[:, t, :], out_offset=None, in_=table[:],
                    in_offset=bass.IndirectOffsetOnAxis(
                        ap=idx_sb[:, k, gi * G + t : gi * G + t + 1],
                        axis=0))
            alpha, nA2 = _fold_step(
                nc, pools, acc, lambda t: ybuf[:, t, 0:256],
                ybuf[:, :, 256], False, alpha, nA2, G)

        if do_bias:
            # mobius_add(NORM * acc, b): alpha *= NORM (nA2 unchanged)
            alpha_s = ch.tile([P, G], f32, tag="st_alpha")
            V.tensor_scalar(out=alpha_s[:], in0=alpha[:], scalar1=NORM,
                            scalar2=None, op0=OP.mult)
            alpha, nA2 = _fold_step(
                nc, pools, acc, lambda t: bias_row[:],
                bn2col, True, alpha_s, nA2, G)

        group_cb(gi, acc, alpha, nA2)


def _activation_store(nc, pools, ti, A_ap, alpha_col, a2_col, scale,
                      dst_dram):
    """expmap0(relu(logmap0(y))) * scale for y = alpha*A; store row+y2.

    a2_col must be the TRUE squared norm ||alpha*A||^2.
    """
    V = nc.vector
    S = nc.scalar
    sb, ch = pools["sb"], pools["ch"]

    z = sb.tile([P, DH], f32, tag="zrelu")
    V.tensor_scalar(out=z[:], in0=A_ap, scalar1=0.0, scalar2=None,
                    op0=OP.max)
    zsq = sb.tile([P, DH], f32, tag="zsq")
    rn2 = ch.tile([P, 1], f32, tag="av_rn2")
    S.activation(zsq[:], z[:], AF.Square, accum_out=rn2[:])
    yn = ch.tile([P, 1], f32, tag="av_yn")
    S.activation(yn[:], a2_col, AF.Sqrt)
    ync = ch.tile([P, 1], f32, tag="av_ync")
    V.tensor_scalar(out=ync[:], in0=yn[:], scalar1=EPS, scalar2=MAXN,
                    op0=OP.max, op1=OP.min)
    s = _atanh_over(nc, ch, ync, 1)
    rnr = ch.tile([P, 1], f32, tag="av_rnr")
    S.activation(rnr[:], rn2[:], AF.Sqrt)
    p1 = ch.tile([P, 1], f32, tag="av_p1")
    V.tensor_tensor(out=p1[:], in0=s[:], in1=alpha_col, op=OP.mult)
    r = ch.tile([P, 1], f32, tag="av_r")
    V.tensor_tensor(out=r[:], in0=p1[:], in1=rnr[:], op=OP.mult)
    gt = ch.tile([P, 1], f32, tag="av_gt")
    S.activation(gt[:], r[:], AF.Tanh)
    rr = ch.tile([P, 1], f32, tag="av_rr")
    V.reciprocal(rr[:], rnr[:])
    gam = ch.tile([P, 1], f32, tag="av_gam")
    V.tensor_tensor(out=gam[:], in0=gt[:], in1=rr[:], op=OP.mult)
    gams = ch.tile([P, 1], f32, tag="av_gams")
    V.tensor_scalar(out=gams[:], in0=gam[:], scalar1=scale, scalar2=None,
                    op0=OP.mult)
    htile = sb.tile([P, 257], f32, tag="htile")
    V.tensor_scalar(out=htile[:, 0:256], in0=z[:], scalar1=gams[:],
                    scalar2=None, op0=OP.mult)
    gg = ch.tile([P, 1], f32, tag="av_gg")
    V.tensor_tensor(out=gg[:], in0=gams[:], in1=gams[:], op=OP.mult)
    V.tensor_tensor(out=htile[:, 256:257], in0=gg[:], in1=rn2[:],
                    op=OP.mult)
    nc.sync.dma_start(dst_dram[ti * P : (ti + 1) * P, 0:257], htile[:])


def _true_a2(nc, ch, alpha, nA2, g):
    aa = ch.tile([P, g], f32, tag="ta_aa")
    nc.vector.tensor_tensor(out=aa[:], in0=alpha[:], in1=alpha[:],
                            op=OP.mult)
    a2 = ch.tile([P, g], f32, tag="ta_a2")
    nc.vector.tensor_tensor(out=a2[:], in0=aa[:], in1=nA2[:], op=OP.mult)
    return a2


def _build_nc():
    nc = bacc.Bacc("TRN2", target_bir_lowering=False, debug=False,
                   num_devices=NCORES)
    feat = nc.dram_tensor("feat", [NSH, DIN], f32, kind="ExternalInput")
    src = nc.dram_tensor("src", [P, TILES * KDEG], i32, kind="ExternalInput")
    sel = nc.dram_tensor("sel", [8, 1], i32, kind="ExternalInput")
    W1 = nc.dram_tensor("W1", [DIN, DH], f32, kind="ExternalInput")
    b1 = nc.dram_tensor("b1", [1, DH], f32, kind="ExternalInput")
    W2 = nc.dram_tensor("W2", [DH, DH], f32, kind="ExternalInput")
    b2 = nc.dram_tensor("b2", [1, DH], f32, kind="ExternalInput")
    WlT = nc.dram_tensor("WlT", [DH, DOUT], f32, kind="ExternalInput")
    bl = nc.dram_tensor("bl", [1, DOUT], f32, kind="ExternalInput")
    out = nc.dram_tensor("out", [8, DOUT], f32, kind="ExternalOutput")
    if KDEBUG:
        dbg_t1 = nc.dram_tensor("dbg_t1", [NSH, ST], f32,
                                kind="ExternalOutput")
        dbg_h1 = nc.dram_tensor("dbg_h1", [NSH, ST], f32,
                                kind="ExternalOutput")
        dbg_h2 = nc.dram_tensor("dbg_h2", [NSH, ST], f32,
                                kind="ExternalOutput")

    with tile.TileContext(nc) as tc:
        with (
            tc.tile_pool(name="sb", bufs=3) as sb,
            tc.tile_pool(name="ch", bufs=3) as ch,
            tc.tile_pool(name="yb", bufs=2) as yb,
            tc.tile_pool(name="ac", bufs=1) as ac,
            tc.tile_pool(name="vg", bufs=1) as vg,
            tc.tile_pool(name="wt", bufs=1) as wt,
            tc.tile_pool(name="ps", bufs=2, space="PSUM") as ps,
            tc.tile_pool(name="psmx", bufs=4, space="PSUM") as psmx,
            tc.tile_pool(name="dr", bufs=1, space="DRAM") as dr,
        ):
            pools = {"sb": sb, "ch": ch, "yb": yb, "ac": ac, "vg": vg}
            ident = wt.tile([P, P], f32, tag="ident")
            make_identity(nc, ident[:])

            # --- weights to SBUF ---
            W1sb = wt.tile([P, 4, DH], f32, tag="W1sb")
            nc.sync.dma_start(
                W1sb[:], W1[:].rearrange("(a p) d -> p a d", p=P))
            W2sb = wt.tile([P, 2, DH], f32, tag="W2sb")
            nc.sync.dma_start(
                W2sb[:], W2[:].rearrange("(a p) d -> p a d", p=P))
            Wlsb = wt.tile([P, 2, DOUT], f32, tag="Wlsb")
            nc.sync.dma_start(
                Wlsb[:], WlT[:].rearrange("(a p) d -> p a d", p=P))
            b1row = wt.tile([1, DH], f32, tag="b1row")
            nc.sync.dma_start(b1row[:], b1[:])
            b1b = wt.tile([P, DH], f32, tag="b1b")
            nc.gpsimd.partition_broadcast(b1b[:], b1row[:])
            b2row = wt.tile([1, DH], f32, tag="b2row")
            nc.sync.dma_start(b2row[:], b2[:])
            b2b = wt.tile([P, DH], f32, tag="b2b")
            nc.gpsimd.partition_broadcast(b2b[:], b2row[:])
            blrow = wt.tile([1, DOUT], f32, tag="blrow")
            nc.sync.dma_start(blrow[:], bl[:])
            blb = wt.tile([8, DOUT], f32, tag="blb")
            nc.gpsimd.partition_broadcast(blb[:], blrow[:], channels=8)

            bscr = wt.tile([P, DH], f32, tag="bscr")
            b1n2 = wt.tile([P, 1], f32, tag="b1n2")
            nc.scalar.activation(bscr[:], b1b[:], AF.Square,
                                 accum_out=b1n2[:])
            bscr2 = wt.tile([P, DH], f32, tag="bscr2")
            b2n2 = wt.tile([P, 1], f32, tag="b2n2")
            nc.scalar.activation(bscr2[:], b2b[:], AF.Square,
                                 accum_out=b2n2[:])
            bscr3 = wt.tile([8, DOUT], f32, tag="bscr3")
            bln2 = wt.tile([8, 1], f32, tag="bln2")
            nc.scalar.activation(bscr3[:], blb[:], AF.Square,
                                 accum_out=bln2[:])

            # --- indices (k-major: [P, KDEG, TILES]) ---
            idx_sb = wt.tile([P, KDEG, TILES], i32, tag="idx")
            nc.sync.dma_start(
                idx_sb[:],
                src[:].rearrange("p (k t) -> p k t", t=TILES))

            # --- DRAM tables ---
            t1sh = dr.tile([NSH, ST], f32, tag="t1sh")
            t1full = dr.tile([N, ST], f32, tag="t1full", addr_space="Shared")
            h1sh = dr.tile([NSH, ST], f32, tag="h1sh")
            h1full = dr.tile([N, ST], f32, tag="h1full", addr_space="Shared")
            h2sh = dr.tile([NSH, ST], f32, tag="h2sh")

            # ================= Phase T: layer-1 transform =================
            for g0 in (range(0, TILES, TG) if STAGE >= 1 else []):
                xn2 = ch.tile([P, TG], f32, tag="tf_xn2")
                mxn2 = ch.tile([P, TG], f32, tag="tf_mxn2")
                pmx_list = []
                for j in range(TG):
                    ti = g0 + j
                    ft = sb.tile([P, DIN], f32, tag="ft")
                    nc.sync.dma_start(ft[:], feat[ti * P : (ti + 1) * P, :])
                    sq = sb.tile([P, DIN], f32, tag="sq")
                    nc.scalar.activation(sq[:], ft[:], AF.Square,
                                         accum_out=xn2[:, j : j + 1])
                    xT = sb.tile([P, 4, P], f32, tag="xT")
                    for c in range(4):
                        pt = ps.tile([P, P], f32, tag="pt")
                        nc.tensor.transpose(
                            out=pt[:], in_=ft[:, c * P : (c + 1) * P],
                            identity=ident[:])
                        nc.vector.tensor_copy(out=xT[:, c, :], in_=pt[:])
                    pmx = psmx.tile([P, DH], f32, tag="pmx")
                    for c in range(4):
                        nc.tensor.matmul(out=pmx[:], lhsT=xT[:, c, :],
                                         rhs=W1sb[:, c, :],
                                         start=(c == 0), stop=(c == 3))
                    msq = sb.tile([P, DH], f32, tag="msq")
                    nc.scalar.activation(msq[:], pmx[:], AF.Square,
                                         accum_out=mxn2[:, j : j + 1])
                    pmx_list.append(pmx)
                # chain (batched over TG tiles): scale = tanh(r)/sqrt(mxn2)
                xnr = ch.tile([P, TG], f32, tag="tf_xnr")
                nc.scalar.activation(xnr[:], xn2[:], AF.Sqrt)
                xn = ch.tile([P, TG], f32, tag="tf_xn")
                nc.vector.tensor_scalar(out=xn[:], in0=xnr[:], scalar1=NORM,
                                        scalar2=EPS, op0=OP.mult, op1=OP.max)
                xnc = ch.tile([P, TG], f32, tag="tf_xnc")
                nc.vector.tensor_scalar(out=xnc[:], in0=xn[:], scalar1=MAXN,
                                        scalar2=None, op0=OP.min)
                at = _atanh_over(nc, ch, xnc, TG)
                mxr = ch.tile([P, TG], f32, tag="tf_mxr")
                nc.scalar.activation(mxr[:], mxn2[:], AF.Sqrt)
                mxn = ch.tile([P, TG], f32, tag="tf_mxn")
                nc.vector.tensor_scalar(out=mxn[:], in0=mxr[:], scalar1=NORM,
                                        scalar2=EPS, op0=OP.mult, op1=OP.max)
                r2 = ch.tile([P, TG], f32, tag="tf_r2")
                nc.vector.tensor_tensor(out=r2[:], in0=mxn[:], in1=at[:],
                                        op=OP.mult)
                th = ch.tile([P, TG], f32, tag="tf_th")
                nc.scalar.activation(th[:], r2[:], AF.Tanh)
                rmx = ch.tile([P, TG], f32, tag="tf_rmx")
                nc.vector.reciprocal(rmx[:], mxr[:])
                srow = ch.tile([P, TG], f32, tag="tf_srow")
                nc.vector.tensor_tensor(out=srow[:], in0=th[:], in1=rmx[:],
                                        op=OP.mult)
                y2r = ch.tile([P, TG], f32, tag="tf_y2r")
                nc.scalar.activation(y2r[:], th[:], AF.Square)
                for j in range(TG):
                    ti = g0 + j
                    ttile = sb.tile([P, 257], f32, tag="ttile")
                    nc.vector.tensor_scalar(
                        out=ttile[:, 0:256], in0=pmx_list[j][:],
                        scalar1=srow[:, j : j + 1], scalar2=None, op0=OP.mult)
                    nc.vector.tensor_copy(out=ttile[:, 256:257],
                                          in_=y2r[:, j : j + 1])
                    nc.sync.dma_start(
                        t1sh[ti * P : (ti + 1) * P, 0:257], ttile[:])

            # ================= AllGather t1 =================
            if STAGE >= 2:
                nc.gpsimd.collective_compute(
                    "AllGather", OP.bypass,
                    replica_groups=[list(range(NCORES))],
                    ins=[t1sh[:]], outs=[t1full[:]])

            # ================= Phase A1: layer-1 aggregation ==============
            if STAGE >= 3:
                def group_cb1(gi, acc, alpha, nA2):
                    a2 = _true_a2(nc, ch, alpha, nA2, G)
                    for t in range(G):
                        _activation_store(nc, pools, gi * G + t,
                                          acc[:, t, 0:256],
                                          alpha[:, t : t + 1],
                                          a2[:, t : t + 1],
                                          NORM, h1sh)

                _aggregate(nc, pools, idx_sb, t1full, b1b, b1n2[:, 0:1],
                           group_cb1)

            # ================= AllGather h1 =================
            if STAGE >= 4:
                nc.gpsimd.collective_compute(
                    "AllGather", OP.bypass,
                    replica_groups=[list(range(NCORES))],
                    ins=[h1sh[:]], outs=[h1full[:]])

                # ==== Phase A2: layer-2 aggregation + W2 matvec + act ====
                def group_cb2(gi, acc, alpha, nA2):
                    a2full = _true_a2(nc, ch, alpha, nA2, G)
                    vgrp = vg.tile([P, G, DH], f32, tag="vgrp")
                    vn2g = ch.tile([P, G], f32, tag="vn2g")
                    for j0 in range(0, G, TG):
                        m2n2 = ch.tile([P, TG], f32, tag="m2_n2")
                        pm_list = []
                        for j in range(TG):
                            t = j0 + j
                            aT = sb.tile([P, 2, P], f32, tag="aT")
                            for c in range(2):
                                pt2 = ps.tile([P, P], f32, tag="pt")
                                nc.tensor.transpose(
                                    out=pt2[:],
                                    in_=acc[:, t, c * P : (c + 1) * P],
                                    identity=ident[:])
                                nc.vector.tensor_copy(out=aT[:, c, :],
                                                      in_=pt2[:])
                            pm2 = psmx.tile([P, DH], f32, tag="pmx")
                            for c in range(2):
                                nc.tensor.matmul(out=pm2[:],
                                                 lhsT=aT[:, c, :],
                                                 rhs=W2sb[:, c, :],
                                                 start=(c == 0),
                                                 stop=(c == 1))
                            ms2 = sb.tile([P, DH], f32, tag="msq")
                            nc.scalar.activation(
                                ms2[:], pm2[:], AF.Square,
                                accum_out=m2n2[:, j : j + 1])
                            pm_list.append(pm2)
                        asl = slice(j0, j0 + TG)
                        xnr2 = ch.tile([P, TG], f32, tag="m2_xnr")
                        nc.scalar.activation(xnr2[:], a2full[:, asl],
                                             AF.Sqrt)
                        xnc2 = ch.tile([P, TG], f32, tag="m2_xnc")
                        nc.vector.tensor_scalar(out=xnc2[:], in0=xnr2[:],
                                                scalar1=EPS, scalar2=MAXN,
                                                op0=OP.max, op1=OP.min)
                        at2 = _atanh_over(nc, ch, xnc2, TG)
                        mxr2 = ch.tile([P, TG], f32, tag="m2_mxr")
                        nc.scalar.activation(mxr2[:], m2n2[:], AF.Sqrt)
                        amx = ch.tile([P, TG], f32, tag="m2_amx")
                        nc.vector.tensor_tensor(out=amx[:],
                                                in0=alpha[:, asl],
                                                in1=mxr2[:], op=OP.mult)
                        amxc = ch.tile([P, TG], f32, tag="m2_amxc")
                        nc.vector.tensor_scalar(out=amxc[:], in0=amx[:],
                                                scalar1=EPS, scalar2=None,
                                                op0=OP.max)
                        r22 = ch.tile([P, TG], f32, tag="m2_r2")
                        nc.vector.tensor_tensor(out=r22[:], in0=amxc[:],
                                                in1=at2[:], op=OP.mult)
                        th2 = ch.tile([P, TG], f32, tag="m2_th")
                        nc.scalar.activation(th2[:], r22[:], AF.Tanh)
                        rmx2 = ch.tile([P, TG], f32, tag="m2_rmx")
                        nc.vector.reciprocal(rmx2[:], mxr2[:])
                        srow2 = ch.tile([P, TG], f32, tag="m2_srow")
                        nc.vector.tensor_tensor(out=srow2[:], in0=th2[:],
                                                in1=rmx2[:], op=OP.mult)
                        nc.scalar.activation(vn2g[:, asl], th2[:],
                                             AF.Square)
                        for j in range(TG):
                            nc.vector.tensor_scalar(
                                out=vgrp[:, j0 + j, :], in0=pm_list[j][:],
                                scalar1=srow2[:, j : j + 1], scalar2=None,
                                op0=OP.mult)
                    # bias fold: mobius_add(NORM * v, b2); rep alpha=NORM
                    alpha2 = ch.tile([P, G], f32, tag="st_alpha")
                    nc.vector.memset(alpha2[:], NORM)
                    alpha2, vn2g2 = _fold_step(
                        nc, pools, vgrp, lambda t: b2b[:],
                        b2n2[:, 0:1], True, alpha2, vn2g, G)
                    a2b = _true_a2(nc, ch, alpha2, vn2g2, G)
                    for t in range(G):
                        _activation_store(nc, pools, gi * G + t,
                                          vgrp[:, t, :],
                                          alpha2[:, t : t + 1],
                                          a2b[:, t : t + 1], 1.0, h2sh)

                _aggregate(nc, pools, idx_sb, h1full, b2b, b2n2[:, 0:1],
                           group_cb2, do_bias=False)

            # ================= Phase D: final classifier =================
            if STAGE >= 5:
                selt = wt.tile([8, 1], i32, tag="selt")
                nc.sync.dma_start(selt[:], sel[:])
                hr = wt.tile([8, 257], f32, tag="hr")
                nc.gpsimd.indirect_dma_start(
                    out=hr[:], out_offset=None, in_=h2sh[:],
                    in_offset=bass.IndirectOffsetOnAxis(ap=selt[:, 0:1],
                                                        axis=0))
                hT = wt.tile([P, 2, 8], f32, tag="hT")
                for c in range(2):
                    pt3 = ps.tile([P, P], f32, tag="pt")
                    nc.tensor.transpose(out=pt3[:, 0:8],
                                        in_=hr[:, c * P : (c + 1) * P],
                                        identity=ident[0:8, 0:8])
                    nc.vector.tensor_copy(out=hT[:, c, :], in_=pt3[:, 0:8])
                pmf = psmx.tile([8, DOUT], f32, tag="pmx")
                for c in range(2):
                    nc.tensor.matmul(out=pmf[:], lhsT=hT[:, c, :],
                                     rhs=Wlsb[:, c, :],
                                     start=(c == 0), stop=(c == 1))
                mfn2 = wt.tile([8, 1], f32, tag="mfn2")
                msf = wt.tile([8, DOUT], f32, tag="msf")
                nc.scalar.activation(msf[:], pmf[:], AF.Square,
                                     accum_out=mfn2[:])
                xnf = wt.tile([8, 1], f32, tag="xnf")
                nc.scalar.activation(xnf[:], hr[:, 256:257], AF.Sqrt)
                xnfc = wt.tile([8, 1], f32, tag="xnfc")
                nc.vector.tensor_scalar(out=xnfc[:], in0=xnf[:], scalar1=EPS,
                                        scalar2=MAXN, op0=OP.max, op1=OP.min)
                atf = _atanh_over(nc, wt, xnfc, 1, p=8)
                mxrf = wt.tile([8, 1], f32, tag="mxrf")
                nc.scalar.activation(mxrf[:], mfn2[:], AF.Sqrt)
                mxnf = wt.tile([8, 1], f32, tag="mxnf")
                nc.vector.tensor_scalar(out=mxnf[:], in0=mxrf[:],
                                        scalar1=EPS, scalar2=None,
                                        op0=OP.max)
                rf2 = wt.tile([8, 1], f32, tag="rf2")
                nc.vector.tensor_tensor(out=rf2[:], in0=mxnf[:], in1=atf[:],
                                        op=OP.mult)
                thf = wt.tile([8, 1], f32, tag="thf")
                nc.scalar.activation(thf[:], rf2[:], AF.Tanh)
                rmxf = wt.tile([8, 1], f32, tag="rmxf")
                nc.vector.reciprocal(rmxf[:], mxrf[:])
                srf = wt.tile([8, 1], f32, tag="srf")
                nc.vector.tensor_tensor(out=srf[:], in0=thf[:], in1=rmxf[:],
                                        op=OP.mult)
                Vf = wt.tile([8, DOUT], f32, tag="Vf")
                nc.vector.tensor_scalar(out=Vf[:], in0=pmf[:],
                                        scalar1=srf[:], scalar2=None,
                                        op0=OP.mult)
                # mobius_add(Vf, bl) on [8, 64]
                x2f = wt.tile([8, 1], f32, tag="x2f")
                nc.vector.tensor_tensor(out=x2f[:], in0=thf[:], in1=thf[:],
                                        op=OP.mult)
                dotf = wt.tile([8, 1], f32, tag="dotf")
                prodf = wt.tile([8, DOUT], f32, tag="prodf")
                nc.vector.scalar_tensor_tensor(
                    out=prodf[:], in0=Vf[:], scalar=1.0, in1=blb[:],
                    op0=OP.mult, op1=OP.mult, accum_out=dotf[:])
                uf = wt.tile([8, 1], f32, tag="uf")
                nc.scalar.activation(uf[:], dotf[:], AF.Copy, bias=1.0,
                                     scale=2.0)
                caf = wt.tile([8, 1], f32, tag="caf")
                nc.vector.tensor_tensor(out=caf[:], in0=uf[:], in1=bln2[:],
                                        op=OP.add)
                t0f = wt.tile([8, 1], f32, tag="t0f")
                nc.vector.tensor_tensor(out=t0f[:], in0=x2f[:], in1=bln2[:],
                                        op=OP.mult)
                denf = wt.tile([8, 1], f32, tag="denf")
                nc.vector.tensor_tensor(out=denf[:], in0=uf[:], in1=t0f[:],
                                        op=OP.add)
                denfc = wt.tile([8, 1], f32, tag="denfc")
                nc.vector.tensor_scalar(out=denfc[:], in0=denf[:],
                                        scalar1=EPS, scalar2=None,
                                        op0=OP.max)
                df = wt.tile([8, 1], f32, tag="df")
                nc.vector.reciprocal(df[:], denfc[:])
                s1f = wt.tile([8, 1], f32, tag="s1f")
                nc.vector.tensor_tensor(out=s1f[:], in0=caf[:], in1=df[:],
                                        op=OP.mult)
                cyf = wt.tile([8, 1], f32, tag="cyf")
                nc.scalar.activation(cyf[:], x2f[:], AF.Copy, bias=1.0,
                                     scale=-1.0)
                s2f = wt.tile([8, 1], f32, tag="s2f")
                nc.vector.tensor_tensor(out=s2f[:], in0=cyf[:], in1=df[:],
                                        op=OP.mult)
                vs1 = wt.tile([8, DOUT], f32, tag="vs1")
                nc.vector.tensor_scalar(out=vs1[:], in0=Vf[:],
                                        scalar1=s1f[:], scalar2=None,
                                        op0=OP.mult)
                outt = wt.tile([8, DOUT], f32, tag="outt")
                nc.vector.scalar_tensor_tensor(
                    out=outt[:], in0=blb[:], scalar=s2f[:], in1=vs1[:],
                    op0=OP.mult, op1=OP.add)
                nc.sync.dma_start(out[:], outt[:])

            if STAGE < 5:
                dumt = wt.tile([8, DOUT], f32, tag="dumt")
                nc.vector.memset(dumt[:], 0.0)
                nc.sync.dma_start(out[:], dumt[:])
            if KDEBUG:
                nc.sync.dma_start(dbg_t1[:], t1sh[:])
                if STAGE >= 3:
                    nc.sync.dma_start(dbg_h1[:], h1sh[:])
                if STAGE >= 4:
                    nc.sync.dma_start(dbg_h2[:], h2sh[:])

    nc.compile()
    return nc


def _get_nc():
    if "nc" not in _NC_CACHE:
        _NC_CACHE["nc"] = _build_nc()
    return _NC_CACHE["nc"]


def kernel(features, W1, b1, W2, b2, Wl, bl, src_idx, to_fetch):
    global LAST_EXEC_NS, LAST_RESULT
    nc = _get_nc()
    features = np.asarray(features, dtype=np.float32)
    src_idx = np.asarray(src_idx, dtype=np.int32)
    to_fetch = np.asarray(to_fetch, dtype=np.int32)
    W1 = np.ascontiguousarray(np.asarray(W1, np.float32))
    b1 = np.asarray(b1, np.float32).reshape(1, DH)
    W2 = np.ascontiguousarray(np.asarray(W2, np.float32))
    b2 = np.asarray(b2, np.float32).reshape(1, DH)
    WlT = np.ascontiguousarray(np.asarray(Wl, np.float32).T)
    bl = np.asarray(bl, np.float32).reshape(1, DOUT)

    in_maps = []
    for c in range(NCORES):
        fsh = np.ascontiguousarray(features[c * NSH : (c + 1) * NSH])
        ssh = src_idx[c * NSH : (c + 1) * NSH]
        ssh = np.ascontiguousarray(
            ssh.reshape(TILES, P, KDEG).transpose(1, 2, 0).reshape(
                P, KDEG * TILES))
        bidx = np.arange(c * 8, (c + 1) * 8, dtype=np.int32)
        selv = (to_fetch[bidx] + bidx * (N // B) - c * NSH).astype(
            np.int32).reshape(8, 1)
        in_maps.append({
            "feat": fsh, "src": ssh, "sel": selv,
            "W1": W1, "b1": b1, "W2": W2, "b2": b2, "WlT": WlT, "bl": bl,
        })
    res = run_bass_kernel_spmd(nc, in_maps, core_ids=list(range(NCORES)),
                               trace=TRACE)
    LAST_RESULT = res
    LAST_EXEC_NS = res.exec_time_ns
    return np.concatenate([res.results[c]["out"] for c in range(NCORES)],
                          axis=0)



# revision 3
# speedup vs baseline: 24.3479x; 24.3479x over previous
"""Hyperbolic GNN classifier on 8 Trainium2 NeuronCores (Bass/Tile).

Only B=64 output rows are consumed (h2[to_fetch + 64*arange]), so the
kernel computes just the dependency cone of those rows: 8 outputs per
core -> 128 layer-1 aggregation instances -> 2048 feature rows. Each
core is fully independent (no collectives): it receives the full
feature/src_idx tables in DRAM and gathers what it needs.

Per core: gather src_idx rows of the 8 selected nodes (-> 128 L1 ids),
gather their src_idx rows (-> [128,16] L2 ids), gather the 2048 feature
rows as 16 tiles of [128, 512], run the W1 mobius_matvec transform per
tile, then a 15-step sequential Mobius fold across the 16 tiles
([128, 256] per step), bias-fold + logmap/relu/expmap activation, a
DRAM roundtrip to regroup [128] instances into [8, 16] fold order,
the 15-step layer-2 fold on [8, 256], W2 mobius_matvec, bias + act,
and the final mobius Linear 256->64. All math fp32, identical to the
reference chain.
"""

import os

import numpy as np

import concourse.bass as bass
import concourse.bacc as bacc
import concourse.mybir as mybir
import concourse.tile as tile
from concourse.bass_utils import run_bass_kernel_spmd
from concourse.masks import make_identity

f32 = mybir.dt.float32
i32 = mybir.dt.int32
OP = mybir.AluOpType
AF = mybir.ActivationFunctionType

NCORES = 8
N = 65536
KDEG = 16
DIN = 512
DH = 256
DOUT = 64
B = 64
P = 128
NPC = B // NCORES          # 8 outputs per core
EPS = 1e-7
MAXN = 1.0 - 1e-5
NORM = float(KDEG) ** -0.5  # 0.25
TG = 4                      # tiles per transform chain sub-batch

TRACE = False
LAST_RESULT = None
LAST_EXEC_NS = None
KDEBUG = bool(int(os.environ.get("KDEBUG", "0")))
_NC_CACHE = {}


def _atanh_over(nc, pool, yn, p, tag):
    """s = arctanh(yn)/yn as [p, 1]; yn pre-clipped to [EPS, MAXN]."""
    V = nc.vector
    S = nc.scalar
    a = pool.tile([p, 1], f32, tag=f"ao_a{tag}")
    V.tensor_scalar(out=a[:], in0=yn[:], scalar1=1.0, scalar2=None,
                    op0=OP.add)
    bm = pool.tile([p, 1], f32, tag=f"ao_b{tag}")
    V.tensor_scalar(out=bm[:], in0=yn[:], scalar1=-1.0, scalar2=1.0,
                    op0=OP.mult, op1=OP.add)
    rb = pool.tile([p, 1], f32, tag=f"ao_c{tag}")
    V.reciprocal(rb[:], bm[:])
    v = pool.tile([p, 1], f32, tag=f"ao_d{tag}")
    V.tensor_tensor(out=v[:], in0=a[:], in1=rb[:], op=OP.mult)
    w = pool.tile([p, 1], f32, tag=f"ao_e{tag}")
    S.activation(w[:], v[:], AF.Ln)
    q = pool.tile([p, 1], f32, tag=f"ao_f{tag}")
    V.reciprocal(q[:], yn[:])
    s = pool.tile([p, 1], f32, tag=f"ao_g{tag}")
    V.tensor_tensor(out=s[:], in0=w[:], in1=q[:], op=OP.mult)
    sh = pool.tile([p, 1], f32, tag=f"ao_h{tag}")
    V.tensor_scalar(out=sh[:], in0=s[:], scalar1=0.5, scalar2=None,
                    op0=OP.mult)
    return sh


def _mstep(nc, pool, acc, y_ap, y2_ap, p, width=DH):
    """acc <- mobius_add(acc, y). acc is [p, width+1]; col `width` holds
    ||acc||^2 and is kept up to date (via ACT square-accum)."""
    V = nc.vector
    S = nc.scalar
    x2 = acc[:, width : width + 1]
    prod = pool.tile([p, width], f32, tag="ms_prod")
    xy = pool.tile([p, 1], f32, tag="ms_xy")
    V.scalar_tensor_tensor(out=prod[:], in0=acc[:, 0:width], scalar=1.0,
                           in1=y_ap, op0=OP.mult, op1=OP.mult,
                           accum_out=xy[:])
    u = pool.tile([p, 1], f32, tag="ms_u")
    S.activation(u[:], xy[:], AF.Copy, bias=1.0, scale=2.0)  # 1+2xy
    can = pool.tile([p, 1], f32, tag="ms_can")
    V.tensor_tensor(out=can[:], in0=u[:], in1=y2_ap, op=OP.add)
    cbn = pool.tile([p, 1], f32, tag="ms_cbn")
    S.activation(cbn[:], x2, AF.Copy, bias=1.0, scale=-1.0)  # 1-x2
    t0 = pool.tile([p, 1], f32, tag="ms_t0")
    V.tensor_tensor(out=t0[:], in0=x2, in1=y2_ap, op=OP.mult)
    den = pool.tile([p, 1], f32, tag="ms_den")
    V.tensor_tensor(out=den[:], in0=u[:], in1=t0[:], op=OP.add)
    denc = pool.tile([p, 1], f32, tag="ms_denc")
    V.tensor_scalar(out=denc[:], in0=den[:], scalar1=EPS, scalar2=None,
                    op0=OP.max)
    r = pool.tile([p, 1], f32, tag="ms_r")
    V.reciprocal(r[:], denc[:])
    # acc = (can*acc + cbn*y) * r
    t1_ = pool.tile([p, width], f32, tag="ms_t1")
    V.tensor_scalar(out=t1_[:], in0=acc[:, 0:width], scalar1=can[:, 0:1],
                    scalar2=None, op0=OP.mult)
    t2_ = pool.tile([p, width], f32, tag="ms_t2")
    V.scalar_tensor_tensor(out=t2_[:], in0=y_ap, scalar=cbn[:, 0:1],
                           in1=t1_[:], op0=OP.mult, op1=OP.add)
    V.tensor_scalar(out=acc[:, 0:width], in0=t2_[:], scalar1=r[:, 0:1],
                    scalar2=None, op0=OP.mult)
    sq = pool.tile([p, width], f32, tag="ms_sq")
    S.activation(sq[:], acc[:, 0:width], AF.Square, accum_out=x2)


def _act_store(nc, pool, acc, out_ap, outn2_ap, scale, p, width=DH):
    """out = scale * expmap0(relu(logmap0(acc))); also writes
    ||out||^2 to outn2_ap. acc [p, width+1] with exact norm col."""
    V = nc.vector
    S = nc.scalar
    z = pool.tile([p, width], f32, tag="as_z")
    V.tensor_scalar(out=z[:], in0=acc[:, 0:width], scalar1=0.0,
                    scalar2=None, op0=OP.max)
    zsq = pool.tile([p, width], f32, tag="as_zsq")
    rn2 = pool.tile([p, 1], f32, tag="as_rn2")
    S.activation(zsq[:], z[:], AF.Square, accum_out=rn2[:])
    yn = pool.tile([p, 1], f32, tag="as_yn")
    S.activation(yn[:], acc[:, width : width + 1], AF.Sqrt)
    ync = pool.tile([p, 1], f32, tag="as_ync")
    V.tensor_scalar(out=ync[:], in0=yn[:], scalar1=EPS, scalar2=MAXN,
                    op0=OP.max, op1=OP.min)
    s = _atanh_over(nc, pool, ync, p, "as")
    rnr0 = pool.tile([p, 1], f32, tag="as_rnr0")
    S.activation(rnr0[:], rn2[:], AF.Sqrt)
    rnr = pool.tile([p, 1], f32, tag="as_rnr")
    V.tensor_scalar(out=rnr[:], in0=rnr0[:], scalar1=EPS, scalar2=None,
                    op0=OP.max)
    rarg = pool.tile([p, 1], f32, tag="as_rarg")
    V.tensor_tensor(out=rarg[:], in0=s[:], in1=rnr[:], op=OP.mult)
    gt = pool.tile([p, 1], f32, tag="as_gt")
    S.activation(gt[:], rarg[:], AF.Tanh)
    rr = pool.tile([p, 1], f32, tag="as_rr")
    V.reciprocal(rr[:], rnr[:])
    gam = pool.tile([p, 1], f32, tag="as_gam")
    V.tensor_tensor(out=gam[:], in0=gt[:], in1=rr[:], op=OP.mult)
    gams = pool.tile([p, 1], f32, tag="as_gams")
    V.tensor_scalar(out=gams[:], in0=gam[:], scalar1=scale, scalar2=None,
                    op0=OP.mult)
    V.tensor_scalar(out=out_ap, in0=z[:], scalar1=gams[:, 0:1],
                    scalar2=None, op0=OP.mult)
    gg = pool.tile([p, 1], f32, tag="as_gg")
    V.tensor_tensor(out=gg[:], in0=gams[:], in1=gams[:], op=OP.mult)
    V.tensor_tensor(out=outn2_ap, in0=gg[:], in1=rn2[:], op=OP.mult)


def _mobius_matvec(nc, pool, ps, psmx, ident, x_ap, xn2_ap, Wsb, nchunk,
                   dout, p, out, outn2):
    """out = mobius_matvec(x, W) on [p, nchunk*128] -> [p, dout];
    outn2 gets ||out||^2 (= tanh(r)^2)."""
    V = nc.vector
    S = nc.scalar
    xT = pool.tile([P, nchunk, p], f32, tag="mv_xT")
    for c in range(nchunk):
        pt = ps.tile([P, P], f32, tag="pt")
        nc.tensor.transpose(out=pt[:, 0:p], in_=x_ap[:, c * P : (c + 1) * P],
                            identity=ident[0:p, 0:p])
        V.tensor_copy(out=xT[:, c, :], in_=pt[:, 0:p])
    pm = psmx.tile([p, dout], f32, tag="pmx")
    for c in range(nchunk):
        nc.tensor.matmul(out=pm[:], lhsT=xT[:, c, :], rhs=Wsb[:, c, :],
                         start=(c == 0), stop=(c == nchunk - 1))
    msq = pool.tile([p, dout], f32, tag="mv_msq")
    mxn2 = pool.tile([p, 1], f32, tag="mv_mxn2")
    S.activation(msq[:], pm[:], AF.Square, accum_out=mxn2[:])
    xnr = pool.tile([p, 1], f32, tag="mv_xnr")
    S.activation(xnr[:], xn2_ap, AF.Sqrt)
    xnc = pool.tile([p, 1], f32, tag="mv_xnc")
    V.tensor_scalar(out=xnc[:], in0=xnr[:], scalar1=EPS, scalar2=MAXN,
                    op0=OP.max, op1=OP.min)
    at = _atanh_over(nc, pool, xnc, p, "mv")
    mxr = pool.tile([p, 1], f32, tag="mv_mxr")
    S.activation(mxr[:], mxn2[:], AF.Sqrt)
    mxn = pool.tile([p, 1], f32, tag="mv_mxn")
    V.tensor_scalar(out=mxn[:], in0=mxr[:], scalar1=EPS, scalar2=None,
                    op0=OP.max)
    rarg = pool.tile([p, 1], f32, tag="mv_rarg")
    V.tensor_tensor(out=rarg[:], in0=mxn[:], in1=at[:], op=OP.mult)
    th = pool.tile([p, 1], f32, tag="mv_th")
    S.activation(th[:], rarg[:], AF.Tanh)
    rmx = pool.tile([p, 1], f32, tag="mv_rmx")
    V.reciprocal(rmx[:], mxr[:])
    srow = pool.tile([p, 1], f32, tag="mv_srow")
    V.tensor_tensor(out=srow[:], in0=th[:], in1=rmx[:], op=OP.mult)
    V.tensor_scalar(out=out, in0=pm[:], scalar1=srow[:, 0:1],
                    scalar2=None, op0=OP.mult)
    S.activation(outn2, th[:], AF.Square)


def _build_nc():
    nc = bacc.Bacc("TRN2", target_bir_lowering=False, debug=False,
                   num_devices=NCORES)
    feat = nc.dram_tensor("feat", [N, DIN], f32, kind="ExternalInput")
    srcs = nc.dram_tensor("srcs", [N, KDEG], i32, kind="ExternalInput")
    sel = nc.dram_tensor("sel", [NPC, 1], i32, kind="ExternalInput")
    W1 = nc.dram_tensor("W1", [DIN, DH], f32, kind="ExternalInput")
    b1 = nc.dram_tensor("b1", [1, DH], f32, kind="ExternalInput")
    W2 = nc.dram_tensor("W2", [DH, DH], f32, kind="ExternalInput")
    b2 = nc.dram_tensor("b2", [1, DH], f32, kind="ExternalInput")
    WlT = nc.dram_tensor("WlT", [DH, DOUT], f32, kind="ExternalInput")
    bl = nc.dram_tensor("bl", [1, DOUT], f32, kind="ExternalInput")
    out = nc.dram_tensor("out", [NPC, DOUT], f32, kind="ExternalOutput")
    if KDEBUG:
        dbg_t1 = nc.dram_tensor("dbg_t1", [KDEG * P, DH + 1], f32,
                                kind="ExternalOutput")
        dbg_h1 = nc.dram_tensor("dbg_h1", [P, DH + 1], f32,
                                kind="ExternalOutput")
        dbg_h2 = nc.dram_tensor("dbg_h2", [NPC, DH + 1], f32,
                                kind="ExternalOutput")
        dbg_s2 = nc.dram_tensor("dbg_s2", [P, KDEG], i32,
                                kind="ExternalOutput")

    with tile.TileContext(nc) as tc:
        with (
            tc.tile_pool(name="sb", bufs=3) as sb,
            tc.tile_pool(name="ch", bufs=4) as ch,
            tc.tile_pool(name="wt", bufs=1) as wt,
            tc.tile_pool(name="yt", bufs=1) as yt,
            tc.tile_pool(name="ps", bufs=2, space="PSUM") as ps,
            tc.tile_pool(name="psmx", bufs=4, space="PSUM") as psmx,
            tc.tile_pool(name="dr", bufs=1, space="DRAM") as dr,
        ):
            ident = wt.tile([P, P], f32, tag="ident")
            make_identity(nc, ident[:])

            # --- weights to SBUF ---
            W1sb = wt.tile([P, 4, DH], f32, tag="W1sb")
            nc.sync.dma_start(
                W1sb[:], W1[:].rearrange("(a p) d -> p a d", p=P))
            W2sb = wt.tile([P, 2, DH], f32, tag="W2sb")
            nc.sync.dma_start(
                W2sb[:], W2[:].rearrange("(a p) d -> p a d", p=P))
            Wlsb = wt.tile([P, 2, DOUT], f32, tag="Wlsb")
            nc.sync.dma_start(
                Wlsb[:], WlT[:].rearrange("(a p) d -> p a d", p=P))
            b1row = wt.tile([1, DH], f32, tag="b1row")
            nc.sync.dma_start(b1row[:], b1[:])
            b1b = wt.tile([P, DH], f32, tag="b1b")
            nc.gpsimd.partition_broadcast(b1b[:], b1row[:])
            b2row = wt.tile([1, DH], f32, tag="b2row")
            nc.sync.dma_start(b2row[:], b2[:])
            b2b = wt.tile([NPC, DH], f32, tag="b2b")
            nc.gpsimd.partition_broadcast(b2b[:], b2row[:], channels=NPC)
            blrow = wt.tile([1, DOUT], f32, tag="blrow")
            nc.sync.dma_start(blrow[:], bl[:])
            blb = wt.tile([NPC, DOUT], f32, tag="blb")
            nc.gpsimd.partition_broadcast(blb[:], blrow[:], channels=NPC)
            bscr = wt.tile([P, DH], f32, tag="bscr")
            b1n2 = wt.tile([P, 1], f32, tag="b1n2")
            nc.scalar.activation(bscr[:], b1b[:], AF.Square,
                                 accum_out=b1n2[:])
            bscr2 = wt.tile([NPC, DH], f32, tag="bscr2")
            b2n2 = wt.tile([NPC, 1], f32, tag="b2n2")
            nc.scalar.activation(bscr2[:], b2b[:], AF.Square,
                                 accum_out=b2n2[:])
            bscr3 = wt.tile([NPC, DOUT], f32, tag="bscr3")
            bln2 = wt.tile([NPC, 1], f32, tag="bln2")
            nc.scalar.activation(bscr3[:], blb[:], AF.Square,
                                 accum_out=bln2[:])

            # --- index gathers: sel -> S1 (128 ids) -> S2 [128, 16] ---
            selt = wt.tile([NPC, 1], i32, tag="selt")
            nc.sync.dma_start(selt[:], sel[:])
            sidx1 = wt.tile([NPC, KDEG], i32, tag="sidx1")
            nc.gpsimd.indirect_dma_start(
                out=sidx1[:], out_offset=None, in_=srcs[:],
                in_offset=bass.IndirectOffsetOnAxis(ap=selt[:, 0:1], axis=0))
            s1d = dr.tile([P, 1], i32, tag="s1d")
            nc.sync.dma_start(
                s1d[:].rearrange("(q k) one -> q (k one)", q=NPC), sidx1[:])
            s1sb = wt.tile([P, 1], i32, tag="s1sb")
            nc.sync.dma_start(s1sb[:], s1d[:])
            sidx2 = wt.tile([P, KDEG], i32, tag="sidx2")
            nc.gpsimd.indirect_dma_start(
                out=sidx2[:], out_offset=None, in_=srcs[:],
                in_offset=bass.IndirectOffsetOnAxis(ap=s1sb[:, 0:1], axis=0))
            if KDEBUG:
                nc.sync.dma_start(dbg_s2[:], sidx2[:])

            # --- feature gathers + layer-1 transform -> ytab [128,16,257]
            ytab = yt.tile([P, KDEG, DH + 1], f32, tag="ytab")
            for g0 in range(0, KDEG, TG):
                xn2 = ch.tile([P, TG], f32, tag="tf_xn2")
                mxn2 = ch.tile([P, TG], f32, tag="tf_mxn2")
                pmx_list = []
                for j in range(TG):
                    k = g0 + j
                    ft = sb.tile([P, DIN], f32, tag="ft")
                    nc.gpsimd.indirect_dma_start(
                        out=ft[:], out_offset=None, in_=feat[:],
                        in_offset=bass.IndirectOffsetOnAxis(
                            ap=sidx2[:, k : k + 1], axis=0))
                    sq = sb.tile([P, DIN], f32, tag="sq")
                    nc.scalar.activation(sq[:], ft[:], AF.Square,
                                         accum_out=xn2[:, j : j + 1])
                    xT = sb.tile([P, 4, P], f32, tag="xT")
                    for c in range(4):
                        pt = ps.tile([P, P], f32, tag="pt")
                        nc.tensor.transpose(
                            out=pt[:], in_=ft[:, c * P : (c + 1) * P],
                            identity=ident[:])
                        nc.vector.tensor_copy(out=xT[:, c, :], in_=pt[:])
                    pmx = psmx.tile([P, DH], f32, tag="pmx")
                    for c in range(4):
                        nc.tensor.matmul(out=pmx[:], lhsT=xT[:, c, :],
                                         rhs=W1sb[:, c, :],
                                         start=(c == 0), stop=(c == 3))
                    msq = sb.tile([P, DH], f32, tag="msq")
                    nc.scalar.activation(msq[:], pmx[:], AF.Square,
                                         accum_out=mxn2[:, j : j + 1])
                    pmx_list.append(pmx)
                # batched mobius_matvec chain over TG tiles
                xnr = ch.tile([P, TG], f32, tag="tf_xnr")
                nc.scalar.activation(xnr[:], xn2[:], AF.Sqrt)
                xn = ch.tile([P, TG], f32, tag="tf_xn")
                nc.vector.tensor_scalar(out=xn[:], in0=xnr[:], scalar1=NORM,
                                        scalar2=EPS, op0=OP.mult, op1=OP.max)
                xnc = ch.tile([P, TG], f32, tag="tf_xnc")
                nc.vector.tensor_scalar(out=xnc[:], in0=xn[:], scalar1=MAXN,
                                        scalar2=None, op0=OP.min)
                # arctanh(xnc)/xnc batched [P, TG]
                aa = ch.tile([P, TG], f32, tag="tf_aa")
                nc.vector.tensor_scalar(out=aa[:], in0=xnc[:], scalar1=1.0,
                                        scalar2=None, op0=OP.add)
                ab = ch.tile([P, TG], f32, tag="tf_ab")
                nc.vector.tensor_scalar(out=ab[:], in0=xnc[:], scalar1=-1.0,
                                        scalar2=1.0, op0=OP.mult, op1=OP.add)
                arb = ch.tile([P, TG], f32, tag="tf_arb")
                nc.vector.reciprocal(arb[:], ab[:])
                av = ch.tile([P, TG], f32, tag="tf_av")
                nc.vector.tensor_tensor(out=av[:], in0=aa[:], in1=arb[:],
                                        op=OP.mult)
                aw = ch.tile([P, TG], f32, tag="tf_aw")
                nc.scalar.activation(aw[:], av[:], AF.Ln)
                aq = ch.tile([P, TG], f32, tag="tf_aq")
                nc.vector.reciprocal(aq[:], xnc[:])
                at = ch.tile([P, TG], f32, tag="tf_at")
                nc.vector.tensor_tensor(out=at[:], in0=aw[:], in1=aq[:],
                                        op=OP.mult)
                ah = ch.tile([P, TG], f32, tag="tf_ah")
                nc.vector.tensor_scalar(out=ah[:], in0=at[:], scalar1=0.5,
                                        scalar2=None, op0=OP.mult)
                mxr = ch.tile([P, TG], f32, tag="tf_mxr")
                nc.scalar.activation(mxr[:], mxn2[:], AF.Sqrt)
                mxn = ch.tile([P, TG], f32, tag="tf_mxn")
                nc.vector.tensor_scalar(out=mxn[:], in0=mxr[:], scalar1=NORM,
                                        scalar2=EPS, op0=OP.mult, op1=OP.max)
                r2 = ch.tile([P, TG], f32, tag="tf_r2")
                nc.vector.tensor_tensor(out=r2[:], in0=mxn[:], in1=ah[:],
                                        op=OP.mult)
                th = ch.tile([P, TG], f32, tag="tf_th")
                nc.scalar.activation(th[:], r2[:], AF.Tanh)
                rmx = ch.tile([P, TG], f32, tag="tf_rmx")
                nc.vector.reciprocal(rmx[:], mxr[:])
                srow = ch.tile([P, TG], f32, tag="tf_srow")
                nc.vector.tensor_tensor(out=srow[:], in0=th[:], in1=rmx[:],
                                        op=OP.mult)
                y2r = ch.tile([P, TG], f32, tag="tf_y2r")
                nc.scalar.activation(y2r[:], th[:], AF.Square)
                for j in range(TG):
                    k = g0 + j
                    nc.vector.tensor_scalar(
                        out=ytab[:, k, 0:DH], in0=pmx_list[j][:],
                        scalar1=srow[:, j : j + 1], scalar2=None, op0=OP.mult)
                    nc.vector.tensor_copy(out=ytab[:, k, DH : DH + 1],
                                          in_=y2r[:, j : j + 1])
            if KDEBUG:
                nc.sync.dma_start(
                    dbg_t1[:].rearrange("(k p) d -> p k d", p=P), ytab[:])

            # --- layer-1 fold (15 steps on [128, 256]) ---
            acc1 = yt.tile([P, DH + 1], f32, tag="acc1")
            nc.vector.tensor_copy(out=acc1[:], in_=ytab[:, 0, :])
            for k in range(1, KDEG):
                _mstep(nc, ch, acc1, ytab[:, k, 0:DH],
                       ytab[:, k, DH : DH + 1], P)
            # rst *= NORM (norm col by NORM^2)
            nc.vector.tensor_scalar(out=acc1[:, 0:DH], in0=acc1[:, 0:DH],
                                    scalar1=NORM, scalar2=None, op0=OP.mult)
            nc.vector.tensor_scalar(out=acc1[:, DH : DH + 1],
                                    in0=acc1[:, DH : DH + 1],
                                    scalar1=NORM * NORM, scalar2=None,
                                    op0=OP.mult)
            _mstep(nc, ch, acc1, b1b[:], b1n2[:, 0:1], P)
            h1t = yt.tile([P, DH + 1], f32, tag="h1t")
            _act_store(nc, ch, acc1, h1t[:, 0:DH], h1t[:, DH : DH + 1],
                       NORM, P)
            if KDEBUG:
                nc.sync.dma_start(dbg_h1[:], h1t[:])

            # --- regroup [128] -> [8, 16] via DRAM roundtrip ---
            h1d = dr.tile([P, DH + 1], f32, tag="h1d")
            nc.sync.dma_start(h1d[:], h1t[:])
            h1r = yt.tile([NPC, KDEG, DH + 1], f32, tag="h1r")
            nc.sync.dma_start(
                h1r[:], h1d[:].rearrange("(q k) d -> q k d", q=NPC))

            # --- layer-2 fold on [8, 256] ---
            acc2 = yt.tile([NPC, DH + 1], f32, tag="acc2")
            nc.vector.tensor_copy(out=acc2[:], in_=h1r[:, 0, :])
            for k in range(1, KDEG):
                _mstep(nc, ch, acc2, h1r[:, k, 0:DH],
                       h1r[:, k, DH : DH + 1], NPC)
            # mobius_matvec W2
            v2 = yt.tile([NPC, DH + 1], f32, tag="v2")
            _mobius_matvec(nc, ch, ps, psmx, ident, acc2[:, 0:DH],
                           acc2[:, DH : DH + 1], W2sb, 2, DH, NPC,
                           v2[:, 0:DH], v2[:, DH : DH + 1])
            nc.vector.tensor_scalar(out=v2[:, 0:DH], in0=v2[:, 0:DH],
                                    scalar1=NORM, scalar2=None, op0=OP.mult)
            nc.vector.tensor_scalar(out=v2[:, DH : DH + 1],
                                    in0=v2[:, DH : DH + 1],
                                    scalar1=NORM * NORM, scalar2=None,
                                    op0=OP.mult)
            _mstep(nc, ch, v2, b2b[:], b2n2[:, 0:1], NPC)
            h2t = yt.tile([NPC, DH + 1], f32, tag="h2t")
            _act_store(nc, ch, v2, h2t[:, 0:DH], h2t[:, DH : DH + 1],
                       1.0, NPC)
            if KDEBUG:
                nc.sync.dma_start(dbg_h2[:], h2t[:])

            # --- final mobius Linear 256 -> 64 + mobius_add(bl) ---
            vf = yt.tile([NPC, DOUT + 1], f32, tag="vf")
            _mobius_matvec(nc, ch, ps, psmx, ident, h2t[:, 0:DH],
                           h2t[:, DH : DH + 1], Wlsb, 2, DOUT, NPC,
                           vf[:, 0:DOUT], vf[:, DOUT : DOUT + 1])
            _mstep(nc, ch, vf, blb[:], bln2[:, 0:1], NPC, width=DOUT)
            outt = wt.tile([NPC, DOUT], f32, tag="outt")
            nc.vector.tensor_copy(out=outt[:], in_=vf[:, 0:DOUT])
            nc.sync.dma_start(out[:], outt[:])

    nc.compile()
    return nc


def _get_nc():
    if "nc" not in _NC_CACHE:
        _NC_CACHE["nc"] = _build_nc()
    return _NC_CACHE["nc"]


def kernel(features, W1, b1, W2, b2, Wl, bl, src_idx, to_fetch):
    global LAST_EXEC_NS, LAST_RESULT
    nc = _get_nc()
    features = np.ascontiguousarray(np.asarray(features, dtype=np.float32))
    src_idx = np.ascontiguousarray(np.asarray(src_idx, dtype=np.int32))
    to_fetch = np.asarray(to_fetch, dtype=np.int32)
    W1 = np.ascontiguousarray(np.asarray(W1, np.float32))
    b1 = np.asarray(b1, np.float32).reshape(1, DH)
    W2 = np.ascontiguousarray(np.asarray(W2, np.float32))
    b2 = np.asarray(b2, np.float32).reshape(1, DH)
    WlT = np.ascontiguousarray(np.asarray(Wl, np.float32).T)
    bl = np.asarray(bl, np.float32).reshape(1, DOUT)

    n_per = N // B
    in_maps = []
    for c in range(NCORES):
        bidx = np.arange(c * NPC, (c + 1) * NPC, dtype=np.int32)
        selv = (to_fetch[bidx] + bidx * n_per).astype(np.int32).reshape(
            NPC, 1)
        in_maps.append({
            "feat": features, "srcs": src_idx, "sel": selv,
            "W1": W1, "b1": b1, "W2": W2, "b2": b2, "WlT": WlT, "bl": bl,
        })
    res = run_bass_kernel_spmd(nc, in_maps, core_ids=list(range(NCORES)),
                               trace=TRACE)
    LAST_RESULT = res
    LAST_EXEC_NS = res.exec_time_ns
    return np.concatenate([res.results[c]["out"] for c in range(NCORES)],
                          axis=0)


# revision 7
# speedup vs baseline: 26.7967x; 1.1006x over previous
"""Hyperbolic GNN classifier on 8 Trainium2 NeuronCores (Bass/Tile).

Only B=64 output rows are consumed (h2[to_fetch + 64*arange]), so the
kernel computes just the dependency cone of those rows: 8 outputs per
core -> 128 layer-1 aggregation instances -> 2048 feature rows. Each
core is fully independent (no collectives): it receives the full
feature/src_idx tables in DRAM and gathers what it needs.

Per core: gather src_idx rows of the 8 selected nodes (-> 128 L1 ids),
gather their src_idx rows (-> [128,16] L2 ids), gather the 2048 feature
rows (cast to fp16) as 16 tiles of [128, 512], run the W1 mobius_matvec
transform per tile (fp16 matmul, fp32 scalars), then a 15-step
sequential Mobius fold across the 16 tiles ([128, 256] fp32 per step),
bias-fold + logmap/relu/expmap activation, a DRAM roundtrip to regroup
[128] instances into [8, 16] fold order, the 15-step layer-2 fold on
[8, 256], W2 mobius_matvec, bias + act, and the final mobius Linear
256->64.
"""

import os

import numpy as np

import concourse.bass as bass
import concourse.bacc as bacc
import concourse.mybir as mybir
import concourse.tile as tile
from concourse.bass_utils import run_bass_kernel_spmd
from concourse.masks import make_identity

f32 = mybir.dt.float32
f16 = mybir.dt.float16
i32 = mybir.dt.int32
OP = mybir.AluOpType
AF = mybir.ActivationFunctionType

NCORES = 8
N = 65536
KDEG = 16
DIN = 512
DH = 256
DOUT = 64
B = 64
P = 128
NPC = B // NCORES          # 8 outputs per core
EPS = 1e-7
MAXN = 1.0 - 1e-5
NORM = float(KDEG) ** -0.5  # 0.25
TG = 4                      # tiles per transform chain sub-batch

TRACE = False
LAST_RESULT = None
LAST_EXEC_NS = None
KDEBUG = bool(int(os.environ.get("KDEBUG", "0")))
_NC_CACHE = {}


def _atanh_over(nc, pool, yn, p, tag):
    """s = arctanh(yn)/yn as [p, 1]; yn pre-clipped to [EPS, MAXN]."""
    V = nc.vector
    S = nc.scalar
    a = pool.tile([p, 1], f32, tag=f"ao_a{tag}")
    V.tensor_scalar(out=a[:], in0=yn[:], scalar1=1.0, scalar2=None,
                    op0=OP.add)
    bm = pool.tile([p, 1], f32, tag=f"ao_b{tag}")
    V.tensor_scalar(out=bm[:], in0=yn[:], scalar1=-1.0, scalar2=1.0,
                    op0=OP.mult, op1=OP.add)
    rb = pool.tile([p, 1], f32, tag=f"ao_c{tag}")
    V.reciprocal(rb[:], bm[:])
    v = pool.tile([p, 1], f32, tag=f"ao_d{tag}")
    V.tensor_tensor(out=v[:], in0=a[:], in1=rb[:], op=OP.mult)
    w = pool.tile([p, 1], f32, tag=f"ao_e{tag}")
    S.activation(w[:], v[:], AF.Ln)
    q = pool.tile([p, 1], f32, tag=f"ao_f{tag}")
    V.reciprocal(q[:], yn[:])
    s = pool.tile([p, 1], f32, tag=f"ao_g{tag}")
    V.tensor_tensor(out=s[:], in0=w[:], in1=q[:], op=OP.mult)
    sh = pool.tile([p, 1], f32, tag=f"ao_h{tag}")
    V.tensor_scalar(out=sh[:], in0=s[:], scalar1=0.5, scalar2=None,
                    op0=OP.mult)
    return sh


def _mstep(nc, pool, acc, y_ap, y2_ap, py2_ap, cbn, p, width=DH):
    """acc <- mobius_add(acc, y). acc is [p, width+1] with col `width`
    = ||acc||^2 (kept exact via ACT square-accum). py2_ap = 1+||y||^2,
    cbn = (1 - ||acc||^2) [p,1] from the previous step. Returns the
    next step's cbn. den's EPS clamp is dropped: den >= (1-xn*yn)^2 and
    all norms here are < 0.5, so it can never bind."""
    V = nc.vector
    S = nc.scalar
    x2 = acc[:, width : width + 1]
    t0 = pool.tile([p, 1], f32, tag="ms_t0")
    V.tensor_tensor(out=t0[:], in0=x2, in1=y2_ap, op=OP.mult)
    prod = pool.tile([p, width], f32, tag="ms_prod")
    xy = pool.tile([p, 1], f32, tag="ms_xy")
    V.scalar_tensor_tensor(out=prod[:], in0=acc[:, 0:width], scalar=1.0,
                           in1=y_ap, op0=OP.mult, op1=OP.mult,
                           accum_out=xy[:])
    # can = 1 + 2xy + y2 on ACT (off critical path)
    can = pool.tile([p, 1], f32, tag="ms_can")
    S.activation(can[:], xy[:], AF.Identity, bias=py2_ap, scale=2.0)
    u = pool.tile([p, 1], f32, tag="ms_u")
    V.tensor_scalar(out=u[:], in0=xy[:], scalar1=2.0, scalar2=1.0,
                    op0=OP.mult, op1=OP.add)
    den = pool.tile([p, 1], f32, tag="ms_den")
    V.tensor_tensor(out=den[:], in0=u[:], in1=t0[:], op=OP.add)
    r = pool.tile([p, 1], f32, tag="ms_r")
    V.reciprocal(r[:], den[:])
    cbr = pool.tile([p, 1], f32, tag="ms_cbr")
    V.tensor_tensor(out=cbr[:], in0=cbn[:], in1=r[:], op=OP.mult)
    t1_ = pool.tile([p, width], f32, tag="ms_t1")
    V.tensor_scalar(out=t1_[:], in0=acc[:, 0:width], scalar1=can[:, 0:1],
                    scalar2=r[:, 0:1], op0=OP.mult, op1=OP.mult)
    V.scalar_tensor_tensor(out=acc[:, 0:width], in0=y_ap,
                           scalar=cbr[:, 0:1], in1=t1_[:], op0=OP.mult,
                           op1=OP.add)
    sq = pool.tile([p, width], f32, tag="ms_sq")
    S.activation(sq[:], acc[:, 0:width], AF.Square, accum_out=x2)
    cbn_n = pool.tile([p, 1], f32, tag="ms_cbn")
    S.activation(cbn_n[:], x2, AF.Copy, bias=1.0, scale=-1.0)
    return cbn_n


def _cbn_of(nc, pool, x2_ap, p, tag="cb0"):
    cbn = pool.tile([p, 1], f32, tag=f"ms_{tag}")
    nc.scalar.activation(cbn[:], x2_ap, AF.Copy, bias=1.0, scale=-1.0)
    return cbn


def _act_store(nc, pool, acc, out_ap, outn2_ap, scale, p, width=DH):
    """out = scale * expmap0(relu(logmap0(acc))); also writes
    ||out||^2 to outn2_ap. acc [p, width+1] with exact norm col."""
    V = nc.vector
    S = nc.scalar
    z = pool.tile([p, width], f32, tag="as_z")
    V.tensor_scalar(out=z[:], in0=acc[:, 0:width], scalar1=0.0,
                    scalar2=None, op0=OP.max)
    zsq = pool.tile([p, width], f32, tag="as_zsq")
    rn2 = pool.tile([p, 1], f32, tag="as_rn2")
    S.activation(zsq[:], z[:], AF.Square, accum_out=rn2[:])
    yn = pool.tile([p, 1], f32, tag="as_yn")
    S.activation(yn[:], acc[:, width : width + 1], AF.Sqrt)
    ync = pool.tile([p, 1], f32, tag="as_ync")
    V.tensor_scalar(out=ync[:], in0=yn[:], scalar1=EPS, scalar2=MAXN,
                    op0=OP.max, op1=OP.min)
    s = _atanh_over(nc, pool, ync, p, "as")
    rnr0 = pool.tile([p, 1], f32, tag="as_rnr0")
    S.activation(rnr0[:], rn2[:], AF.Sqrt)
    rnr = pool.tile([p, 1], f32, tag="as_rnr")
    V.tensor_scalar(out=rnr[:], in0=rnr0[:], scalar1=EPS, scalar2=None,
                    op0=OP.max)
    rarg = pool.tile([p, 1], f32, tag="as_rarg")
    V.tensor_tensor(out=rarg[:], in0=s[:], in1=rnr[:], op=OP.mult)
    gt = pool.tile([p, 1], f32, tag="as_gt")
    S.activation(gt[:], rarg[:], AF.Tanh)
    rr = pool.tile([p, 1], f32, tag="as_rr")
    V.reciprocal(rr[:], rnr[:])
    gam = pool.tile([p, 1], f32, tag="as_gam")
    V.tensor_tensor(out=gam[:], in0=gt[:], in1=rr[:], op=OP.mult)
    gams = pool.tile([p, 1], f32, tag="as_gams")
    V.tensor_scalar(out=gams[:], in0=gam[:], scalar1=scale, scalar2=None,
                    op0=OP.mult)
    V.tensor_scalar(out=out_ap, in0=z[:], scalar1=gams[:, 0:1],
                    scalar2=None, op0=OP.mult)
    gg = pool.tile([p, 1], f32, tag="as_gg")
    V.tensor_tensor(out=gg[:], in0=gams[:], in1=gams[:], op=OP.mult)
    V.tensor_tensor(out=outn2_ap, in0=gg[:], in1=rn2[:], op=OP.mult)


def _mobius_matvec(nc, pool, ps, psmx, identsm, x_ap, xn2_ap, Wsb, nchunk,
                   dout, p, out, outn2):
    """out = mobius_matvec(x, W) on [p, nchunk*128] -> [p, dout];
    outn2 gets ||out||^2 (= tanh(r)^2). Wsb is fp16."""
    V = nc.vector
    S = nc.scalar
    xT = pool.tile([P, nchunk, p], f16, tag="mv_xT")
    for c in range(nchunk):
        pt = ps.tile([P, P], f32, tag="pt")
        nc.tensor.transpose(out=pt[:, 0:p], in_=x_ap[:, c * P : (c + 1) * P],
                            identity=identsm[0:p, 0:p])
        V.tensor_copy(out=xT[:, c, :], in_=pt[:, 0:p])
    pm = psmx.tile([p, dout], f32, tag="pmx")
    for c in range(nchunk):
        nc.tensor.matmul(out=pm[:], lhsT=xT[:, c, :], rhs=Wsb[:, c, :],
                         start=(c == 0), stop=(c == nchunk - 1))
    msq = pool.tile([p, dout], f32, tag="mv_msq")
    mxn2 = pool.tile([p, 1], f32, tag="mv_mxn2")
    S.activation(msq[:], pm[:], AF.Square, accum_out=mxn2[:])
    xnr = pool.tile([p, 1], f32, tag="mv_xnr")
    S.activation(xnr[:], xn2_ap, AF.Sqrt)
    xnc = pool.tile([p, 1], f32, tag="mv_xnc")
    V.tensor_scalar(out=xnc[:], in0=xnr[:], scalar1=EPS, scalar2=MAXN,
                    op0=OP.max, op1=OP.min)
    at = _atanh_over(nc, pool, xnc, p, "mv")
    mxr = pool.tile([p, 1], f32, tag="mv_mxr")
    S.activation(mxr[:], mxn2[:], AF.Sqrt)
    mxn = pool.tile([p, 1], f32, tag="mv_mxn")
    V.tensor_scalar(out=mxn[:], in0=mxr[:], scalar1=EPS, scalar2=None,
                    op0=OP.max)
    rarg = pool.tile([p, 1], f32, tag="mv_rarg")
    V.tensor_tensor(out=rarg[:], in0=mxn[:], in1=at[:], op=OP.mult)
    th = pool.tile([p, 1], f32, tag="mv_th")
    S.activation(th[:], rarg[:], AF.Tanh)
    rmx = pool.tile([p, 1], f32, tag="mv_rmx")
    V.reciprocal(rmx[:], mxr[:])
    srow = pool.tile([p, 1], f32, tag="mv_srow")
    V.tensor_tensor(out=srow[:], in0=th[:], in1=rmx[:], op=OP.mult)
    V.tensor_scalar(out=out, in0=pm[:], scalar1=srow[:, 0:1],
                    scalar2=None, op0=OP.mult)
    S.activation(outn2, th[:], AF.Square)


def _build_nc():
    nc = bacc.Bacc("TRN2", target_bir_lowering=False, debug=False,
                   num_devices=NCORES)
    feat = nc.dram_tensor("feat", [N, DIN], f32, kind="ExternalInput")
    srcs = nc.dram_tensor("srcs", [N, KDEG], i32, kind="ExternalInput")
    sel = nc.dram_tensor("sel", [NPC, 1], i32, kind="ExternalInput")
    W1 = nc.dram_tensor("W1", [DIN, DH], f32, kind="ExternalInput")
    b1 = nc.dram_tensor("b1", [1, DH], f32, kind="ExternalInput")
    W2 = nc.dram_tensor("W2", [DH, DH], f32, kind="ExternalInput")
    b2 = nc.dram_tensor("b2", [1, DH], f32, kind="ExternalInput")
    WlT = nc.dram_tensor("WlT", [DH, DOUT], f32, kind="ExternalInput")
    bl = nc.dram_tensor("bl", [1, DOUT], f32, kind="ExternalInput")
    out = nc.dram_tensor("out", [NPC, DOUT], f32, kind="ExternalOutput")
    if KDEBUG:
        dbg_t1 = nc.dram_tensor("dbg_t1", [KDEG * P, DH + 1], f32,
                                kind="ExternalOutput")
        dbg_h1 = nc.dram_tensor("dbg_h1", [P, DH + 1], f32,
                                kind="ExternalOutput")
        dbg_h2 = nc.dram_tensor("dbg_h2", [NPC, DH + 1], f32,
                                kind="ExternalOutput")
        dbg_s2 = nc.dram_tensor("dbg_s2", [P, KDEG], i32,
                                kind="ExternalOutput")

    with tile.TileContext(nc) as tc:
        with (
            tc.tile_pool(name="sb", bufs=4) as sb,
            tc.tile_pool(name="ch", bufs=4) as ch,
            tc.tile_pool(name="wt", bufs=1) as wt,
            tc.tile_pool(name="yt", bufs=1) as yt,
            tc.tile_pool(name="ps", bufs=2, space="PSUM") as ps,
            tc.tile_pool(name="psmx", bufs=4, space="PSUM") as psmx,
            tc.tile_pool(name="dr", bufs=1, space="DRAM") as dr,
        ):
            # --- index chain first (scalar-engine HWDGE ring) ---
            selt = wt.tile([NPC, 1], i32, tag="selt")
            nc.scalar.dma_start(selt[:], sel[:])
            sidx1 = wt.tile([NPC, KDEG], i32, tag="sidx1")
            nc.gpsimd.indirect_dma_start(
                out=sidx1[:], out_offset=None, in_=srcs[:],
                in_offset=bass.IndirectOffsetOnAxis(ap=selt[:, 0:1], axis=0))
            s1d = dr.tile([P, 1], i32, tag="s1d")
            nc.scalar.dma_start(
                s1d[:].rearrange("(q k) one -> q (k one)", q=NPC), sidx1[:])
            s1sb = wt.tile([P, 1], i32, tag="s1sb")
            nc.scalar.dma_start(s1sb[:], s1d[:])
            sidx2 = wt.tile([P, KDEG], i32, tag="sidx2")
            nc.gpsimd.indirect_dma_start(
                out=sidx2[:], out_offset=None, in_=srcs[:],
                in_offset=bass.IndirectOffsetOnAxis(ap=s1sb[:, 0:1], axis=0))
            if KDEBUG:
                nc.sync.dma_start(dbg_s2[:], sidx2[:])

            ident = wt.tile([P, P], f32, tag="ident")
            make_identity(nc, ident[:])
            identh = wt.tile([P, P], f16, tag="identh")
            nc.vector.tensor_copy(out=identh[:], in_=ident[:])

            # --- weights to SBUF (fp16 for matmuls) ---
            W1sb = wt.tile([P, 4, DH], f16, tag="W1sb")
            nc.gpsimd.dma_start(
                W1sb[:], W1[:].rearrange("(a p) d -> p a d", p=P))
            W2sb = wt.tile([P, 2, DH], f16, tag="W2sb")
            nc.gpsimd.dma_start(
                W2sb[:], W2[:].rearrange("(a p) d -> p a d", p=P))
            Wlsb = wt.tile([P, 2, DOUT], f16, tag="Wlsb")
            nc.gpsimd.dma_start(
                Wlsb[:], WlT[:].rearrange("(a p) d -> p a d", p=P))
            b1row = wt.tile([1, DH], f32, tag="b1row")
            nc.sync.dma_start(b1row[:], b1[:])
            b1b = wt.tile([P, DH], f32, tag="b1b")
            nc.gpsimd.partition_broadcast(b1b[:], b1row[:])
            b2row = wt.tile([1, DH], f32, tag="b2row")
            nc.sync.dma_start(b2row[:], b2[:])
            b2b = wt.tile([NPC, DH], f32, tag="b2b")
            nc.gpsimd.partition_broadcast(b2b[:], b2row[:], channels=NPC)
            blrow = wt.tile([1, DOUT], f32, tag="blrow")
            nc.sync.dma_start(blrow[:], bl[:])
            blb = wt.tile([NPC, DOUT], f32, tag="blb")
            nc.gpsimd.partition_broadcast(blb[:], blrow[:], channels=NPC)
            bscr = wt.tile([P, DH], f32, tag="bscr")
            b1n2 = wt.tile([P, 1], f32, tag="b1n2")
            nc.scalar.activation(bscr[:], b1b[:], AF.Square,
                                 accum_out=b1n2[:])
            pb1n2 = wt.tile([P, 1], f32, tag="pb1n2")
            nc.scalar.activation(pb1n2[:], b1n2[:], AF.Copy, bias=1.0)
            bscr2 = wt.tile([NPC, DH], f32, tag="bscr2")
            b2n2 = wt.tile([NPC, 1], f32, tag="b2n2")
            nc.scalar.activation(bscr2[:], b2b[:], AF.Square,
                                 accum_out=b2n2[:])
            pb2n2 = wt.tile([NPC, 1], f32, tag="pb2n2")
            nc.scalar.activation(pb2n2[:], b2n2[:], AF.Copy, bias=1.0)
            bscr3 = wt.tile([NPC, DOUT], f32, tag="bscr3")
            bln2 = wt.tile([NPC, 1], f32, tag="bln2")
            nc.scalar.activation(bscr3[:], blb[:], AF.Square,
                                 accum_out=bln2[:])
            pbln2 = wt.tile([NPC, 1], f32, tag="pbln2")
            nc.scalar.activation(pbln2[:], bln2[:], AF.Copy, bias=1.0)

            # --- feature gathers (fp16 cast) + layer-1 transform ---
            ytiles = [yt.tile([P, DH + 1], f32, name=f"ytile{k}",
                              tag=f"y{k}")
                      for k in range(KDEG)]
            py2t = wt.tile([P, KDEG], f32, tag="py2t")  # 1 + y2 per k
            for g0 in range(0, KDEG, TG):
                xn2 = ch.tile([P, TG], f32, tag="tf_xn2")
                mxn2 = ch.tile([P, TG], f32, tag="tf_mxn2")
                pmx_list = []
                for j in range(TG):
                    k = g0 + j
                    ft = sb.tile([P, DIN], f16, tag="ft")
                    nc.gpsimd.indirect_dma_start(
                        out=ft[:], out_offset=None, in_=feat[:],
                        in_offset=bass.IndirectOffsetOnAxis(
                            ap=sidx2[:, k : k + 1], axis=0))
                    sq = sb.tile([P, DIN], f32, tag="sq")
                    nc.scalar.activation(sq[:], ft[:], AF.Square,
                                         accum_out=xn2[:, j : j + 1])
                    xT = sb.tile([P, 4, P], f16, tag="xT")
                    for c in range(4):
                        pt = ps.tile([P, P], f16, tag="pth")
                        nc.tensor.transpose(
                            out=pt[:], in_=ft[:, c * P : (c + 1) * P],
                            identity=identh[:])
                        nc.vector.tensor_copy(out=xT[:, c, :], in_=pt[:])
                    pmx = psmx.tile([P, DH], f32, tag="pmx")
                    for c in range(4):
                        nc.tensor.matmul(out=pmx[:], lhsT=xT[:, c, :],
                                         rhs=W1sb[:, c, :],
                                         start=(c == 0), stop=(c == 3))
                    msq = sb.tile([P, DH], f32, tag="msq")
                    nc.scalar.activation(msq[:], pmx[:], AF.Square,
                                         accum_out=mxn2[:, j : j + 1])
                    pmx_list.append(pmx)
                # batched mobius_matvec chain over TG tiles
                xnr = ch.tile([P, TG], f32, tag="tf_xnr")
                nc.scalar.activation(xnr[:], xn2[:], AF.Sqrt)
                xn = ch.tile([P, TG], f32, tag="tf_xn")
                nc.vector.tensor_scalar(out=xn[:], in0=xnr[:], scalar1=NORM,
                                        scalar2=EPS, op0=OP.mult, op1=OP.max)
                xnc = ch.tile([P, TG], f32, tag="tf_xnc")
                nc.vector.tensor_scalar(out=xnc[:], in0=xn[:], scalar1=MAXN,
                                        scalar2=None, op0=OP.min)
                # arctanh(xnc)/xnc batched [P, TG]
                aa = ch.tile([P, TG], f32, tag="tf_aa")
                nc.vector.tensor_scalar(out=aa[:], in0=xnc[:], scalar1=1.0,
                                        scalar2=None, op0=OP.add)
                ab = ch.tile([P, TG], f32, tag="tf_ab")
                nc.vector.tensor_scalar(out=ab[:], in0=xnc[:], scalar1=-1.0,
                                        scalar2=1.0, op0=OP.mult, op1=OP.add)
                arb = ch.tile([P, TG], f32, tag="tf_arb")
                nc.vector.reciprocal(arb[:], ab[:])
                av = ch.tile([P, TG], f32, tag="tf_av")
                nc.vector.tensor_tensor(out=av[:], in0=aa[:], in1=arb[:],
                                        op=OP.mult)
                aw = ch.tile([P, TG], f32, tag="tf_aw")
                nc.scalar.activation(aw[:], av[:], AF.Ln)
                aq = ch.tile([P, TG], f32, tag="tf_aq")
                nc.vector.reciprocal(aq[:], xnc[:])
                at = ch.tile([P, TG], f32, tag="tf_at")
                nc.vector.tensor_tensor(out=at[:], in0=aw[:], in1=aq[:],
                                        op=OP.mult)
                ah = ch.tile([P, TG], f32, tag="tf_ah")
                nc.vector.tensor_scalar(out=ah[:], in0=at[:], scalar1=0.5,
                                        scalar2=None, op0=OP.mult)
                mxr = ch.tile([P, TG], f32, tag="tf_mxr")
                nc.scalar.activation(mxr[:], mxn2[:], AF.Sqrt)
                mxn = ch.tile([P, TG], f32, tag="tf_mxn")
                nc.vector.tensor_scalar(out=mxn[:], in0=mxr[:], scalar1=NORM,
                                        scalar2=EPS, op0=OP.mult, op1=OP.max)
                r2 = ch.tile([P, TG], f32, tag="tf_r2")
                nc.vector.tensor_tensor(out=r2[:], in0=mxn[:], in1=ah[:],
                                        op=OP.mult)
                th = ch.tile([P, TG], f32, tag="tf_th")
                nc.scalar.activation(th[:], r2[:], AF.Tanh)
                rmx = ch.tile([P, TG], f32, tag="tf_rmx")
                nc.vector.reciprocal(rmx[:], mxr[:])
                srow = ch.tile([P, TG], f32, tag="tf_srow")
                nc.vector.tensor_tensor(out=srow[:], in0=th[:], in1=rmx[:],
                                        op=OP.mult)
                y2r = ch.tile([P, TG], f32, tag="tf_y2r")
                nc.scalar.activation(y2r[:], th[:], AF.Square)
                nc.vector.tensor_scalar(out=py2t[:, g0 : g0 + TG],
                                        in0=y2r[:], scalar1=1.0,
                                        scalar2=None, op0=OP.add)
                for j in range(TG):
                    k = g0 + j
                    nc.vector.tensor_scalar(
                        out=ytiles[k][:, 0:DH], in0=pmx_list[j][:],
                        scalar1=srow[:, j : j + 1], scalar2=None, op0=OP.mult)
                    nc.vector.tensor_copy(out=ytiles[k][:, DH : DH + 1],
                                          in_=y2r[:, j : j + 1])
            if KDEBUG:
                for k in range(KDEG):
                    nc.sync.dma_start(dbg_t1[k * P : (k + 1) * P, :],
                                      ytiles[k][:])

            # --- layer-1 fold (15 steps on [128, 256]) ---
            acc1 = yt.tile([P, DH + 1], f32, tag="acc1")
            nc.vector.tensor_copy(out=acc1[:], in_=ytiles[0][:])
            cbn = _cbn_of(nc, ch, acc1[:, DH : DH + 1], P)
            for k in range(1, KDEG):
                cbn = _mstep(nc, ch, acc1, ytiles[k][:, 0:DH],
                             ytiles[k][:, DH : DH + 1],
                             py2t[:, k : k + 1], cbn, P)
            # rst *= NORM (norm col by NORM^2)
            nc.vector.tensor_scalar(out=acc1[:, 0:DH], in0=acc1[:, 0:DH],
                                    scalar1=NORM, scalar2=None, op0=OP.mult)
            nc.vector.tensor_scalar(out=acc1[:, DH : DH + 1],
                                    in0=acc1[:, DH : DH + 1],
                                    scalar1=NORM * NORM, scalar2=None,
                                    op0=OP.mult)
            cbn = _cbn_of(nc, ch, acc1[:, DH : DH + 1], P, tag="cb1")
            _mstep(nc, ch, acc1, b1b[:], b1n2[:, 0:1], pb1n2[:, 0:1],
                   cbn, P)
            h1t = yt.tile([P, DH + 1], f32, tag="h1t")
            _act_store(nc, ch, acc1, h1t[:, 0:DH], h1t[:, DH : DH + 1],
                       NORM, P)
            if KDEBUG:
                nc.sync.dma_start(dbg_h1[:], h1t[:])

            # --- regroup [128] -> [8, 16] via DRAM roundtrip ---
            h1d = dr.tile([P, DH + 1], f32, tag="h1d")
            nc.scalar.dma_start(h1d[:], h1t[:])
            h1r = yt.tile([NPC, KDEG, DH + 1], f32, tag="h1r")
            nc.scalar.dma_start(
                h1r[:], h1d[:].rearrange("(q k) d -> q k d", q=NPC))

            # --- layer-2 fold on [8, 256] ---
            py2b = wt.tile([NPC, KDEG], f32, tag="py2b")
            nc.vector.tensor_scalar(out=py2b[:], in0=h1r[:, :, DH],
                                    scalar1=1.0, scalar2=None, op0=OP.add)
            acc2 = yt.tile([NPC, DH + 1], f32, tag="acc2")
            nc.vector.tensor_copy(out=acc2[:], in_=h1r[:, 0, :])
            cbn = _cbn_of(nc, ch, acc2[:, DH : DH + 1], NPC, tag="cb2")
            for k in range(1, KDEG):
                cbn = _mstep(nc, ch, acc2, h1r[:, k, 0:DH],
                             h1r[:, k, DH : DH + 1],
                             py2b[:, k : k + 1], cbn, NPC)
            # mobius_matvec W2
            v2 = yt.tile([NPC, DH + 1], f32, tag="v2")
            _mobius_matvec(nc, ch, ps, psmx, ident, acc2[:, 0:DH],
                           acc2[:, DH : DH + 1], W2sb, 2, DH, NPC,
                           v2[:, 0:DH], v2[:, DH : DH + 1])
            nc.vector.tensor_scalar(out=v2[:, 0:DH], in0=v2[:, 0:DH],
                                    scalar1=NORM, scalar2=None, op0=OP.mult)
            nc.vector.tensor_scalar(out=v2[:, DH : DH + 1],
                                    in0=v2[:, DH : DH + 1],
                                    scalar1=NORM * NORM, scalar2=None,
                                    op0=OP.mult)
            cbn = _cbn_of(nc, ch, v2[:, DH : DH + 1], NPC, tag="cb3")
            _mstep(nc, ch, v2, b2b[:], b2n2[:, 0:1], pb2n2[:, 0:1],
                   cbn, NPC)
            h2t = yt.tile([NPC, DH + 1], f32, tag="h2t")
            _act_store(nc, ch, v2, h2t[:, 0:DH], h2t[:, DH : DH + 1],
                       1.0, NPC)
            if KDEBUG:
                nc.sync.dma_start(dbg_h2[:], h2t[:])

            # --- final mobius Linear 256 -> 64 + mobius_add(bl) ---
            vf = yt.tile([NPC, DOUT + 1], f32, tag="vf")
            _mobius_matvec(nc, ch, ps, psmx, ident, h2t[:, 0:DH],
                           h2t[:, DH : DH + 1], Wlsb, 2, DOUT, NPC,
                           vf[:, 0:DOUT], vf[:, DOUT : DOUT + 1])
            cbn = _cbn_of(nc, ch, vf[:, DOUT : DOUT + 1], NPC, tag="cb4")
            _mstep(nc, ch, vf, blb[:], bln2[:, 0:1], pbln2[:, 0:1],
                   cbn, NPC, width=DOUT)
            outt = wt.tile([NPC, DOUT], f32, tag="outt")
            nc.vector.tensor_copy(out=outt[:], in_=vf[:, 0:DOUT])
            nc.sync.dma_start(out[:], outt[:])

    nc.compile()
    return nc


def _get_nc():
    if "nc" not in _NC_CACHE:
        _NC_CACHE["nc"] = _build_nc()
    return _NC_CACHE["nc"]


def kernel(features, W1, b1, W2, b2, Wl, bl, src_idx, to_fetch):
    global LAST_EXEC_NS, LAST_RESULT
    nc = _get_nc()
    features = np.ascontiguousarray(np.asarray(features, dtype=np.float32))
    src_idx = np.ascontiguousarray(np.asarray(src_idx, dtype=np.int32))
    to_fetch = np.asarray(to_fetch, dtype=np.int32)
    W1 = np.ascontiguousarray(np.asarray(W1, np.float32))
    b1 = np.asarray(b1, np.float32).reshape(1, DH)
    W2 = np.ascontiguousarray(np.asarray(W2, np.float32))
    b2 = np.asarray(b2, np.float32).reshape(1, DH)
    WlT = np.ascontiguousarray(np.asarray(Wl, np.float32).T)
    bl = np.asarray(bl, np.float32).reshape(1, DOUT)

    n_per = N // B
    in_maps = []
    for c in range(NCORES):
        bidx = np.arange(c * NPC, (c + 1) * NPC, dtype=np.int32)
        selv = (to_fetch[bidx] + bidx * n_per).astype(np.int32).reshape(
            NPC, 1)
        in_maps.append({
            "feat": features, "srcs": src_idx, "sel": selv,
            "W1": W1, "b1": b1, "W2": W2, "b2": b2, "WlT": WlT, "bl": bl,
        })
    res = run_bass_kernel_spmd(nc, in_maps, core_ids=list(range(NCORES)),
                               trace=TRACE)
    LAST_RESULT = res
    LAST_EXEC_NS = res.exec_time_ns
    return np.concatenate([res.results[c]["out"] for c in range(NCORES)],
                          axis=0)
